# revision 7
# baseline (speedup 1.0000x reference)
"""AuxLossFreeMoE TRN2 kernel: 16-expert top-2 sigmoid-gated MoE + shared expert.

Strategy (8 NeuronCores, SPMD single program):
  - Routing (sigmoid gating + top-2 + weight normalization) is computed on host
    with the exact same jax CPU ops as the reference: the random centroids
    saturate the sigmoid, producing thousands of exact ties broken by expert
    index, so any approximate device sigmoid (ACT LUT) flips selections.
    Routing is 0.13% of total FLOPs; all FFN compute runs on device.
  - Expert-parallel FFN with static load balancing: each core gets three
    "pieces" of capacity [768, 384, 128] token-slots (1280 slots/core).
    Expert token lists are carved into these pieces (hot experts split across
    cores), so every core does identical work.
  - Each core gathers its tokens from a replicated x (indirect DMA), runs the
    expert SwiGLU in fp32r (full PE rate), scales rows by combine weight, and
    scatters rows into a destination-sorted send buffer.
  - AllToAll moves contributions to token-owner cores; owners gather their two
    contributions per token, add the shared expert output, and write out.
  - Shared expert is sequence-parallel: each core computes shared SwiGLU for
    its own 512 tokens.
"""

import os
import numpy as np

B, S, H = 4, 1024, 2048
E = 16
TOPK = 2
I = 1024
ISH = 2048
RATIO = 0.1
EPS = 1e-9
T = B * S
NC = 8
P = 128
TOWN = T // NC  # 512 tokens owned per core
PIECE_SIZES = (768, 384, 128)
CAP = sum(PIECE_SIZES)  # 1280 slots per core
N_TILES = CAP // P  # 10
KC_H = H // P    # 16
M_I = I // P     # 8
M_ISH = ISH // P  # 16
DUMMY_TOK = T  # extra zero row in x_pad
BIG = 10 ** 9

_COMPILED = {}


def _enable_jax_cache():
    import jax
    try:
        cache_dir = os.environ.get("KERNEL_JAX_CACHE", "/tmp/jax_moe_cache")
        jax.config.update("jax_compilation_cache_dir", cache_dir)
        jax.config.update("jax_persistent_cache_min_compile_time_secs", 0.0)
    except Exception:
        pass


def _host_routing(x, centroids, gate_bias):
    """Bit-identical routing to the reference (jax CPU ops)."""
    import jax
    import jax.numpy as jnp
    cpu = jax.devices("cpu")[0]
    with jax.default_device(cpu):
        xj = jax.device_put(np.asarray(x), cpu)
        cj = jax.device_put(np.asarray(centroids), cpu)
        gj = jax.device_put(np.asarray(gate_bias), cpu)
        aff = jax.nn.sigmoid(jnp.einsum('bsh,eh->bse', xj, cj))
        biased = aff + gj
        _, top_idx = jax.lax.top_k(biased, TOPK)
        top_aff = jnp.take_along_axis(aff, top_idx, axis=-1)
        weights = top_aff / (top_aff.sum(-1, keepdims=True) + EPS)
    top_idx = np.asarray(top_idx).reshape(T, TOPK).astype(np.int64)
    weights = np.asarray(weights).reshape(T, TOPK).astype(np.float32)
    return top_idx, weights


def _assign_pieces(counts):
    """Carve expert token lists into pieces of sizes 768/384/128 (8 of each),
    then pack one piece of each size per core, co-locating same-expert pieces
    to minimize weight traffic. Returns per-core piece lists
    [(expert, offset_in_expert_list, realcount, size), ...] ordered [A,B,C]."""
    avail = {768: 8, 384: 8, 128: 8}
    pieces = {768: [], 384: [], 128: []}
    order = np.argsort(-np.asarray(counts), kind="stable")
    for e in order:
        rem = int(counts[e])
        off = 0
        if rem == 0:
            continue
        while rem > 0:
            if rem > 384 and avail[768] > 0:
                sz = 768
            elif rem > 128 and avail[384] > 0:
                sz = 384
            elif rem <= 128 and avail[128] > 0:
                sz = 128
            elif avail[384] > 0:
                sz = 384
            elif avail[768] > 0:
                sz = 768
            else:
                raise RuntimeError("piece inventory exhausted; routing distribution unexpected")
            avail[sz] -= 1
            take = min(rem, sz)
            pieces[sz].append((int(e), off, take, sz))
            off += take
            rem -= take
    # dummy pieces for unused inventory
    for sz in (768, 384, 128):
        while avail[sz] > 0:
            pieces[sz].append((0, 0, 0, sz))
            avail[sz] -= 1
    # pack cores: one piece of each size; prefer same-expert grouping
    cores = []
    used_b = [False] * 8
    used_c = [False] * 8
    for a in pieces[768]:
        grp = [a]
        be = next((j for j, bp in enumerate(pieces[384])
                   if not used_b[j] and bp[2] > 0 and bp[0] == a[0]), None)
        if be is None:
            be = next(j for j, _ in enumerate(pieces[384]) if not used_b[j])
        used_b[be] = True
        grp.append(pieces[384][be])
        exps = {a[0], pieces[384][be][0]}
        ce = next((j for j, cp in enumerate(pieces[128])
                   if not used_c[j] and cp[2] > 0 and cp[0] in exps), None)
        if ce is None:
            ce = next(j for j, _ in enumerate(pieces[128]) if not used_c[j])
        used_c[ce] = True
        grp.append(pieces[128][ce])
        cores.append(grp)
    return cores


def _build_program():
    """Build the SPMD Bass program (same for all cores)."""
    import concourse.bass as bass
    import concourse.mybir as mybir
    import concourse.tile as tile
    from concourse import bacc
    from concourse.masks import make_identity

    dt = mybir.dt
    AF = mybir.ActivationFunctionType
    ALU = mybir.AluOpType

    SCAP = _build_program.SCAP
    SEND_ROWS = NC * SCAP

    nc = bacc.Bacc("TRN2", target_bir_lowering=False, num_devices=NC)

    f32, f32r, i32 = dt.float32, dt.float32r, dt.int32

    x_pad = nc.dram_tensor("x_pad", [T + 1, H], f32, kind="ExternalInput")
    tok_ids = nc.dram_tensor("tok_ids", [N_TILES, P], i32, kind="ExternalInput")
    wslot = nc.dram_tensor("wslot", [N_TILES, P], f32, kind="ExternalInput")
    send_pos = nc.dram_tensor("send_pos", [N_TILES, P], i32, kind="ExternalInput")
    recv_idx = nc.dram_tensor("recv_idx", [2, TOWN // P, P], i32, kind="ExternalInput")
    wg_in = nc.dram_tensor("wg_in", [3, M_I, P, KC_H, P], f32, kind="ExternalInput")
    wu_in = nc.dram_tensor("wu_in", [3, M_I, P, KC_H, P], f32, kind="ExternalInput")
    wd_in = nc.dram_tensor("wd_in", [3, M_I, P, H], f32, kind="ExternalInput")
    wgs_in = nc.dram_tensor("wgs_in", [M_ISH, P, KC_H, P], f32, kind="ExternalInput")
    wus_in = nc.dram_tensor("wus_in", [M_ISH, P, KC_H, P], f32, kind="ExternalInput")
    wds_in = nc.dram_tensor("wds_in", [M_ISH, P, H], f32, kind="ExternalInput")
    xT_own = nc.dram_tensor("xT_own", [KC_H, P, TOWN], f32, kind="ExternalInput")

    out_own = nc.dram_tensor("out_own", [TOWN, H], f32, kind="ExternalOutput")

    send_buf = nc.dram_tensor("send_buf", [SEND_ROWS, H], f32)
    recv_buf = nc.dram_tensor("recv_buf", [SEND_ROWS, H], f32)

    # piece -> (local tile offset, number of slot tiles, matmul blocks)
    piece_tiles = [sz // P for sz in PIECE_SIZES]
    piece_tile_off = [0, 6, 9]
    piece_blocks = {0: [(0, 512), (512, 256)], 1: [(0, 384)], 2: [(0, 128)]}

    with tile.TileContext(nc) as tc:
        with (
            tc.tile_pool(name="const", bufs=1) as constp,
            tc.tile_pool(name="big", bufs=1) as bigp,
            tc.tile_pool(name="io", bufs=2) as iop,
        ):
            ident = constp.tile([P, P], f32)
            make_identity(nc, ident[:])

            shared_tok = bigp.tile([P, TOWN // P, H], f32, name="shared_tok")
            n_hb = H // 512

            # ---------------- shared expert (own 512 tokens) ----------------
            with (
                tc.tile_pool(name="shbig", bufs=1) as shbig,
                tc.tile_pool(name="shw", bufs=2) as shw,
                tc.tile_pool(name="shps", bufs=1, space="PSUM") as psp,
            ):
                xTo = shbig.tile([P, KC_H, TOWN], f32r, name="xTo")
                nc.sync.dma_start(xTo[:], xT_own.rearrange("kc p t -> p kc t").bitcast(f32r))

                hs = shbig.tile([P, M_ISH, TOWN], f32r, name="hs")
                for m in range(M_ISH):
                    wgs_t = shw.tile([P, KC_H, P], f32r, name="wgs_t", tag="wgs_t")
                    wus_t = shw.tile([P, KC_H, P], f32r, name="wus_t", tag="wus_t")
                    nc.sync.dma_start(wgs_t[:], wgs_in[m].bitcast(f32r))
                    nc.sync.dma_start(wus_t[:], wus_in[m].bitcast(f32r))
                    psg = psp.tile([P, TOWN], f32, name="psg", tag="psg")
                    psu = psp.tile([P, TOWN], f32, name="psu", tag="psu")
                    for kc in range(KC_H):
                        nc.tensor.matmul(psg[:], wgs_t[:, kc, :], xTo[:, kc, :],
                                         start=(kc == 0), stop=(kc == KC_H - 1))
                    for kc in range(KC_H):
                        nc.tensor.matmul(psu[:], wus_t[:, kc, :], xTo[:, kc, :],
                                         start=(kc == 0), stop=(kc == KC_H - 1))
                    sg = shw.tile([P, TOWN], f32r, name="sg", tag="sg")
                    nc.scalar.activation(sg[:], psg[:], AF.Silu)
                    nc.vector.tensor_mul(hs[:, m, :], sg[:], psu[:])

                # shared down-projection, output token-major directly
                for hb in range(n_hb):
                    ps_sh = [psp.tile([P, 512], f32, name=f"ps_sh{tt}", tag=f"ps_sh{tt}")
                             for tt in range(TOWN // P)]
                    for ic in range(M_ISH):
                        wds_t = shw.tile([P, 512], f32r, name="wds_t", tag="wds_t")
                        nc.sync.dma_start(wds_t[:], wds_in[ic][:, hb * 512:(hb + 1) * 512].bitcast(f32r))
                        for tt in range(TOWN // P):
                            nc.tensor.matmul(ps_sh[tt][:], hs[:, ic, tt * P:(tt + 1) * P],
                                             wds_t[:], start=(ic == 0), stop=(ic == M_ISH - 1))
                    for tt in range(TOWN // P):
                        nc.scalar.activation(shared_tok[:, tt, hb * 512:(hb + 1) * 512],
                                             ps_sh[tt][:], AF.Copy, scale=RATIO)

            # ---------------- routed experts: 3 pieces ----------------
            with (
                tc.tile_pool(name="rtbig", bufs=1) as rtbig,
                tc.tile_pool(name="rtw", bufs=2) as rtw,
                tc.tile_pool(name="rtwork", bufs=2) as work,
            ):
                for p_i in range(3):
                    n_t = piece_tiles[p_i]
                    t_off = piece_tile_off[p_i]

                    up_ps = tc.tile_pool(name=f"upps{p_i}", bufs=1, space="PSUM")
                    psp = up_ps.__enter__()
                    tr_ps = tc.tile_pool(name=f"trps{p_i}", bufs=2, space="PSUM")
                    pstp = tr_ps.__enter__()

                    # gather + transpose to feature-major
                    xgT = rtbig.tile([P, KC_H, 768], f32r, name="xgT", tag="xgT")
                    wts = []
                    sidx = []
                    for st in range(n_t):
                        idx_t = iop.tile([P, 1], i32, name="idx_t", tag="idx_t")
                        nc.sync.dma_start(idx_t[:], tok_ids[t_off + st][:, None])
                        w_t = constp.tile([P, 1], f32, name=f"w_t{p_i}_{st}", tag=f"w_t{t_off + st}")
                        nc.sync.dma_start(w_t[:], wslot[t_off + st][:, None])
                        wts.append(w_t)
                        si_t = constp.tile([P, 1], i32, name=f"si_t{p_i}_{st}", tag=f"si_t{t_off + st}")
                        nc.sync.dma_start(si_t[:], send_pos[t_off + st][:, None])
                        sidx.append(si_t)
                        xg = work.tile([P, H], f32, name="xg", tag="xg")
                        nc.gpsimd.indirect_dma_start(
                            out=xg[:], out_offset=None, in_=x_pad[:, :],
                            in_offset=bass.IndirectOffsetOnAxis(ap=idx_t[:, :1], axis=0))
                        for kc in range(KC_H):
                            pst = pstp.tile([P, P], f32, name="pst", tag="pst")
                            nc.tensor.transpose(pst[:], xg[:, kc * P:(kc + 1) * P], ident[:])
                            nc.scalar.activation(xgT[:, kc, st * P:(st + 1) * P], pst[:], AF.Copy)

                    # up/gate projections -> h [i, slots] f32r
                    h = rtbig.tile([P, M_I, 768], f32r, name="h", tag="h")
                    for m in range(M_I):
                        wg_t = rtw.tile([P, KC_H, P], f32r, name="wg_t", tag="wg_t")
                        wu_t = rtw.tile([P, KC_H, P], f32r, name="wu_t", tag="wu_t")
                        nc.sync.dma_start(wg_t[:], wg_in[p_i, m].bitcast(f32r))
                        nc.sync.dma_start(wu_t[:], wu_in[p_i, m].bitcast(f32r))
                        for (b0, bn) in piece_blocks[p_i]:
                            psg2 = psp.tile([P, 512], f32, name="psg2", tag="psg")
                            psu2 = psp.tile([P, 512], f32, name="psu2", tag="psu")
                            for kc in range(KC_H):
                                nc.tensor.matmul(psg2[:, :bn], wg_t[:, kc, :],
                                                 xgT[:, kc, b0:b0 + bn],
                                                 start=(kc == 0), stop=(kc == KC_H - 1))
                            for kc in range(KC_H):
                                nc.tensor.matmul(psu2[:, :bn], wu_t[:, kc, :],
                                                 xgT[:, kc, b0:b0 + bn],
                                                 start=(kc == 0), stop=(kc == KC_H - 1))
                            sg2 = work.tile([P, 512], f32r, name="sg2", tag="sg2")
                            nc.scalar.activation(sg2[:, :bn], psg2[:, :bn], AF.Silu)
                            nc.vector.tensor_mul(h[:, m, b0:b0 + bn], sg2[:, :bn], psu2[:, :bn])

                    # down projection, token-major out; scale; scatter to send_buf
                    tr_ps.__exit__(None, None, None)
                    up_ps.__exit__(None, None, None)
                    dn_ps = tc.tile_pool(name=f"dnps{p_i}", bufs=1, space="PSUM")
                    dpsp = dn_ps.__enter__()
                    for hb in range(n_hb):
                        ps_d = [dpsp.tile([P, 512], f32, name=f"ps_d{st}", tag=f"ps_d{st}")
                                for st in range(n_t)]
                        for ic in range(M_I):
                            wd_t = rtw.tile([P, 512], f32r, name="wd_t", tag="wd_t")
                            nc.sync.dma_start(wd_t[:], wd_in[p_i, ic][:, hb * 512:(hb + 1) * 512].bitcast(f32r))
                            for st in range(n_t):
                                nc.tensor.matmul(ps_d[st][:], h[:, ic, st * P:(st + 1) * P],
                                                 wd_t[:], start=(ic == 0), stop=(ic == M_I - 1))
                        for st in range(n_t):
                            y_blk = work.tile([P, 512], f32, name="y_blk", tag="y_blk")
                            nc.vector.tensor_scalar_mul(y_blk[:], ps_d[st][:], wts[st][:, :1])
                            nc.gpsimd.indirect_dma_start(
                                out=send_buf[:, :], in_=y_blk[:],
                                out_offset=bass.IndirectOffsetOnAxis(ap=sidx[st][:, :1], axis=0),
                                in_offset=None,
                                element_offset=hb * 512,
                                bounds_check=SEND_ROWS - 1,
                                oob_is_err=False)
                    dn_ps.__exit__(None, None, None)

            # ---------------- all-to-all combine ----------------
            nc.gpsimd.collective_compute(
                "AllToAll",
                mybir.AluOpType.bypass,
                replica_groups=[list(range(NC))],
                ins=[send_buf[:, :].opt()],
                outs=[recv_buf[:, :].opt()],
            )

            with tc.tile_pool(name="cmb", bufs=2) as cmb:
                for tt in range(TOWN // P):
                    i1 = iop.tile([P, 1], i32, name="i1", tag="i1")
                    i2 = iop.tile([P, 1], i32, name="i2", tag="i2")
                    nc.sync.dma_start(i1[:], recv_idx[0, tt][:, None])
                    nc.sync.dma_start(i2[:], recv_idx[1, tt][:, None])
                    g1 = cmb.tile([P, H], f32, name="g1", tag="g1")
                    g2 = cmb.tile([P, H], f32, name="g2", tag="g2")
                    nc.gpsimd.indirect_dma_start(
                        out=g1[:], out_offset=None, in_=recv_buf[:, :],
                        in_offset=bass.IndirectOffsetOnAxis(ap=i1[:, :1], axis=0))
                    nc.gpsimd.indirect_dma_start(
                        out=g2[:], out_offset=None, in_=recv_buf[:, :],
                        in_offset=bass.IndirectOffsetOnAxis(ap=i2[:, :1], axis=0))
                    nc.vector.tensor_add(g1[:], g1[:], g2[:])
                    nc.vector.tensor_add(g1[:], g1[:], shared_tok[:, tt, :])
                    nc.sync.dma_start(out_own[tt * P:(tt + 1) * P, :], g1[:])

    nc.finalize()
    return nc


def prepare_in_maps(x, centroids, gate_bias, wg_s, wu_s, wd_s, wg, wu, wd):
    x = np.ascontiguousarray(np.asarray(x, dtype=np.float32))
    wg = np.asarray(wg, dtype=np.float32)
    wu = np.asarray(wu, dtype=np.float32)
    wd = np.asarray(wd, dtype=np.float32)

    top_idx, weights = _host_routing(x, centroids, gate_bias)

    # expert token lists in token order
    lists = [[] for _ in range(E)]
    wvals = [[] for _ in range(E)]
    for t in range(T):
        for k in range(TOPK):
            e = int(top_idx[t, k])
            lists[e].append(t)
            wvals[e].append(weights[t, k])
    counts = [len(l) for l in lists]
    cores = _assign_pieces(counts)

    # per-core slot tables
    tok_ids = np.full((NC, N_TILES, P), DUMMY_TOK, dtype=np.int32)
    wslot = np.zeros((NC, N_TILES, P), dtype=np.float32)
    piece_expert = np.zeros((NC, 3), dtype=np.int64)
    for c in range(NC):
        loc = 0
        for pi, (e, off, cnt, sz) in enumerate(cores[c]):
            piece_expert[c, pi] = e
            for j in range(cnt):
                t = lists[e][off + j]
                tok_ids[c, (loc + j) // P, (loc + j) % P] = t
                wslot[c, (loc + j) // P, (loc + j) % P] = wvals[e][off + j]
            loc += sz

    # send positions / recv indices
    cnt_co = np.zeros((NC, NC), dtype=np.int64)
    contrib = [[] for _ in range(T)]  # (core, pos) per contribution
    for c in range(NC):
        for loc in range(CAP):
            t = int(tok_ids[c, loc // P, loc % P])
            if t == DUMMY_TOK:
                continue
            o = t // TOWN
            pos = cnt_co[c, o]
            cnt_co[c, o] += 1
            contrib[t].append((c, int(pos)))
    SCAP = int(((cnt_co.max() + 15) // 16) * 16)
    # destination row = owner * SCAP + pos
    send_pos_arr = np.full((NC, N_TILES, P), BIG, dtype=np.int32)
    cnt_co2 = np.zeros((NC, NC), dtype=np.int64)
    for c in range(NC):
        for loc in range(CAP):
            t = int(tok_ids[c, loc // P, loc % P])
            if t == DUMMY_TOK:
                continue
            o = t // TOWN
            pos = cnt_co2[c, o]
            cnt_co2[c, o] += 1
            send_pos_arr[c, loc // P, loc % P] = o * SCAP + pos

    recv_idx = np.zeros((NC, 2, TOWN // P, P), dtype=np.int32)
    for t in range(T):
        o = t // TOWN
        tl = t % TOWN
        assert len(contrib[t]) == 2, (t, contrib[t])
        for k, (c, pos) in enumerate(contrib[t]):
            recv_idx[o, k, tl // P, tl % P] = c * SCAP + pos

    # weight tensors, matmul-ready tiling
    def tile_up(w2d, mm):  # [H, mm*128] -> [mm, 128, KC_H, 128]
        return np.ascontiguousarray(
            w2d.reshape(KC_H, P, mm, P).transpose(2, 1, 0, 3))

    def tile_dn(w2d, mm):  # [mm*128, H] -> [mm, 128, H]
        return np.ascontiguousarray(w2d.reshape(mm, P, H))

    wg_t = np.zeros((NC, 3, M_I, P, KC_H, P), dtype=np.float32)
    wu_t = np.zeros((NC, 3, M_I, P, KC_H, P), dtype=np.float32)
    wd_t = np.zeros((NC, 3, M_I, P, H), dtype=np.float32)
    done = {}
    for c in range(NC):
        for pi, (e, off, cnt, sz) in enumerate(cores[c]):
            if cnt == 0:
                continue
            if e not in done:
                done[e] = (tile_up(wg[e], M_I), tile_up(wu[e], M_I), tile_dn(wd[e], M_I))
            wg_t[c, pi], wu_t[c, pi], wd_t[c, pi] = done[e]

    wgs_t = tile_up(np.asarray(wg_s, np.float32), M_ISH)
    wus_t = tile_up(np.asarray(wu_s, np.float32), M_ISH)
    wds_t = tile_dn(np.asarray(wd_s, np.float32), M_ISH)

    x_flat = x.reshape(T, H)
    x_pad = np.vstack([x_flat, np.zeros((1, H), np.float32)])

    in_maps = []
    for c in range(NC):
        xo = np.ascontiguousarray(
            x_flat[c * TOWN:(c + 1) * TOWN].T.reshape(KC_H, P, TOWN))
        in_maps.append({
            "x_pad": x_pad,
            "tok_ids": tok_ids[c],
            "wslot": wslot[c],
            "send_pos": send_pos_arr[c],
            "recv_idx": recv_idx[c],
            "wg_in": wg_t[c],
            "wu_in": wu_t[c],
            "wd_in": wd_t[c],
            "wgs_in": wgs_t,
            "wus_in": wus_t,
            "wds_in": wds_t,
            "xT_own": xo,
        })

    return in_maps, SCAP


def get_program(scap):
    key = ("moe", scap)
    if key not in _COMPILED:
        _build_program.SCAP = scap
        _COMPILED[key] = _build_program()
    return _COMPILED[key]


def kernel(x, centroids, gate_bias, wg_s, wu_s, wd_s, wg, wu, wd):
    _enable_jax_cache()
    from concourse.bass_utils import run_bass_kernel_spmd

    in_maps, scap = prepare_in_maps(x, centroids, gate_bias, wg_s, wu_s, wd_s, wg, wu, wd)
    nc = get_program(scap)
    res = run_bass_kernel_spmd(nc, in_maps, core_ids=list(range(NC)))
    out = np.concatenate([res.results[c]["out_own"] for c in range(NC)], axis=0)
    return out.reshape(B, S, H)
